# revision 1
# baseline (speedup 1.0000x reference)
"""Dilated window attention (W=[4,8,16], R=[1,2,4]) on 8 Trainium2 NeuronCores.

Strategy (per core; b*h = 32 shards, 4 per core):
  * All three dilation groups have L=4 sub-sampled positions per window of
    w = 4*r, so every window is a 4x4 attention over d=64.
  * Unified raw-position strips: a strip is 128 consecutive sequence positions.
    For every group the scores S^T[k,q] = K.Q^T are computed over the SAME
    128x128 raw strip on TensorE (d on partitions), with a group-specific
    low-rank (1+windows) constant mask matmul accumulated into the same PSUM
    group: valid (k,q) pairs (same window, both on the group's dilation grid)
    get +0, everything else -240, which exp() turns into 0. Masks are exact in
    bf16 (-16*15 and 16*15 factorizations).
  * Q,K are cast f32->bf16 (SWDGE cast-DMA into SBUF, partition-adjacent so
    descriptors coalesce), packed bh-pair-interleaved into a DRAM scratch
    [pos, 2*64], and bulk-transposed by the DMA xbar into resident [128, 8192]
    tiles (pair member m at partitions 64m..64m+63). A PSUM accumulation group
    must keep all matmuls in one row-group range, so the mask constants are
    replicated at partition bases 0 and 64.
  * attn^T = exp(0.125*S^T) on ScalarE straight PSUM->SBUF, bf16.
  * AV + softmax denominator in one matmul: lhsT = attn^T, rhs = shared V tile
    (bf16, cast on load) with a leading ones column, so Z lands per-partition
    beside the output. rz = reciprocal(Z) * vmask_g, where vmask_g is a
    per-partition constant = w_g on the group's dilation grid and 0 elsewhere
    (w = softmax(alpha)); this both normalizes and zeroes the off-grid junk
    columns.
  * Because every group's output tile lives in the SAME raw-position partition
    layout, the three weighted contributions are summed on-chip (VectorE mult
    + GpSimd adds) and stored with ONE plain contiguous DMA. No DRAM
    accumulate pass, no dilated gathers: every DMA in the kernel is
    descriptor-coalescible (>=512B partition-adjacent chunks).
"""
import numpy as np

B, H, S, D = 2, 16, 8192, 64
N_CORES = 8
BH_PER_CORE = (B * H) // N_CORES      # 4 -> 2 bh-pairs
N_PAIRS = BH_PER_CORE // 2
RS = [1, 2, 4]                        # within-window stride per group
CHUNK = 2048                          # cast/transpose pipelining chunk (positions)
MRANK = [33, 17, 9]                   # mask rank per group (1 + windows/strip)
MOFF = [0, 33, 50]                    # row offset of each group's mask block
MTOT = 59

_CACHE = {}


def _make_masks(ml_dtypes):
    # group g: valid (k,q) iff same (4r)-window and k,q both on the r-grid.
    # mask_g = -240*J + 240*sum_c u_c u_c^T, factored exactly in bf16.
    mk_rows, mq_rows = [], []
    for r in RS:
        w = 4 * r
        nwin = 128 // w
        U = np.zeros((nwin, 128), np.float32)
        for c in range(nwin):
            U[c, w * c:w * (c + 1):r] = 1.0
        mk_rows.append(np.concatenate(
            [-16.0 * np.ones((1, 128), np.float32), 16.0 * U], 0))
        mq_rows.append(np.concatenate(
            [15.0 * np.ones((1, 128), np.float32), 15.0 * U], 0))
    mask_k = np.concatenate(mk_rows, 0)              # [59, 128]
    mask_q = np.tile(np.concatenate(mq_rows, 0), (1, 4))  # [59, 512]
    return (mask_k.astype(ml_dtypes.bfloat16), mask_q.astype(ml_dtypes.bfloat16))


def _make_vmasks(w):
    # [128, 3]: column g = w_g on partitions on the r-grid, else 0
    vm = np.zeros((128, 3), np.float32)
    for g, r in enumerate(RS):
        vm[::r, g] = w[g]
    return vm


def _build(reps=1):
    import concourse.bacc as bacc
    import concourse.tile as tile
    from concourse import mybir

    F32 = mybir.dt.float32
    BF16 = mybir.dt.bfloat16

    nc = bacc.Bacc("TRN2", target_bir_lowering=False, debug=False,
                   num_devices=N_CORES)
    q = nc.dram_tensor("q", [BH_PER_CORE, S, D], F32, kind="ExternalInput")
    k = nc.dram_tensor("k", [BH_PER_CORE, S, D], F32, kind="ExternalInput")
    v = nc.dram_tensor("v", [BH_PER_CORE, S, D], F32, kind="ExternalInput")
    mask_k = nc.dram_tensor("mask_k", [MTOT, 128], BF16, kind="ExternalInput")
    mask_q = nc.dram_tensor("mask_q", [MTOT, 512], BF16, kind="ExternalInput")
    vmasks = nc.dram_tensor("vmasks", [128, 3], F32, kind="ExternalInput")
    out = nc.dram_tensor("out", [BH_PER_CORE, S, D], F32, kind="ExternalOutput")

    scr_q = nc.dram_tensor("scr_q", [N_PAIRS, S, 128], BF16)
    scr_k = nc.dram_tensor("scr_k", [N_PAIRS, S, 128], BF16)

    with tile.TileContext(nc) as tc:
        with tc.tile_pool(name="const", bufs=1) as constp, \
             tc.tile_pool(name="cast", bufs=4) as castp, \
             tc.tile_pool(name="qt", bufs=2) as qtp, \
             tc.tile_pool(name="vaug", bufs=4) as vaugp, \
             tc.tile_pool(name="attn", bufs=6) as attnp, \
             tc.tile_pool(name="outp", bufs=6) as outp, \
             tc.tile_pool(name="rz", bufs=6) as rzp, \
             tc.tile_pool(name="stp", bufs=4, space="PSUM") as stp, \
             tc.tile_pool(name="o2p", bufs=4, space="PSUM") as o2p:

            # constants: per-group mask tiles, replicated at bases 0 and 64
            # (matmul weights must start at partition base 0/32/64)
            mks, mqs = [], []
            for g in range(3):
                mkg = constp.tile([64 + MRANK[g], 128], BF16, tag=f"mk{g}")
                mqg = constp.tile([64 + MRANK[g], 512], BF16, tag=f"mq{g}")
                for mb in (0, 64):
                    nc.sync.dma_start(
                        out=mkg[mb:mb + MRANK[g], :],
                        in_=mask_k[MOFF[g]:MOFF[g] + MRANK[g], :])
                    nc.sync.dma_start(
                        out=mqg[mb:mb + MRANK[g], :],
                        in_=mask_q[MOFF[g]:MOFF[g] + MRANK[g], :])
                mks.append(mkg)
                mqs.append(mqg)
            vm = constp.tile([128, 3], F32)
            nc.sync.dma_start(out=vm[:], in_=vmasks[:])

            # f32 -> bf16 casts, pair-interleaved in SBUF, stored to scratch
            for rep in range(reps):
                for pair in range(N_PAIRS):
                    for src, scr in ((q, scr_q), (k, scr_k)):
                        for c0 in range(0, S, CHUNK):
                            ct = castp.tile([128, CHUNK // 128, 2, 64], BF16,
                                            tag="cast")
                            for m in range(2):
                                nc.gpsimd.dma_start(
                                    out=ct[:, :, m, :],
                                    in_=src[2 * pair + m, c0:c0 + CHUNK, :]
                                        .rearrange("(j p) d -> p j d", p=128))
                            nc.sync.dma_start(
                                out=scr[pair, c0:c0 + CHUNK, :].rearrange(
                                    "(j p) (m d) -> p j m d", p=128, m=2),
                                in_=ct[:])

            for rep in range(reps):
              for pair in range(N_PAIRS):
                qt = qtp.tile([128, S], BF16, tag="qt")
                kt = qtp.tile([128, S], BF16, tag="kt")
                for src, dst in ((scr_q, qt), (scr_k, kt)):
                    for c0 in range(0, S, CHUNK):
                        nc.sync.dma_start(out=dst[:, c0:c0 + CHUNK],
                                          in_=src[pair, c0:c0 + CHUNK, :],
                                          transpose=True)

                for m in range(2):
                    bh = 2 * pair + m
                    for mt in range(S // 512):
                        p0 = 512 * mt
                        if mt % 4 == 0:
                            vaug4 = vaugp.tile([128, 16, 66], BF16, tag="vaug")
                            nc.gpsimd.dma_start(
                                out=vaug4[:, :, 1:65],
                                in_=v[bh, p0:p0 + 2048, :].rearrange(
                                    "(s p) d -> p s d", s=16, p=128))
                            nc.vector.memset(vaug4[:, :, 0], 1.0)
                        vaug = vaug4[:, 4 * (mt % 4):4 * (mt % 4) + 4, :]

                        ot = outp.tile([128, 256], F32, tag="ot")
                        otv = ot[:].rearrange("p (s d) -> p s d", d=64)
                        for g in range(3):
                            st = stp.tile([128, 512], F32, tag="st",
                                          space="PSUM")
                            nc.tensor.matmul(
                                out=st[:],
                                lhsT=mks[g][64 * m:64 * m + MRANK[g], :],
                                rhs=mqs[g][64 * m:64 * m + MRANK[g], :],
                                start=True, stop=False)
                            for s4 in range(4):
                                c0 = p0 + 128 * s4
                                nc.tensor.matmul(
                                    out=st[:, 128 * s4:128 * s4 + 128],
                                    lhsT=kt[64 * m:64 * m + 64, c0:c0 + 128],
                                    rhs=qt[64 * m:64 * m + 64, c0:c0 + 128],
                                    start=False, stop=(s4 == 3))
                            attn = attnp.tile([128, 512], BF16, tag="attn")
                            nc.scalar.activation(
                                out=attn[:], in_=st[:],
                                func=mybir.ActivationFunctionType.Exp,
                                scale=float(D) ** -0.5)
                            o2 = o2p.tile([128, 260], F32, tag="o2",
                                          space="PSUM")
                            for s4 in range(4):
                                nc.tensor.matmul(
                                    out=o2[:, 65 * s4:65 * s4 + 65],
                                    lhsT=attn[:, 128 * s4:128 * s4 + 128],
                                    rhs=vaug[:, s4, 0:65],
                                    start=True, stop=True)
                            o2v = o2[:].rearrange("p (s c) -> p s c", c=65)
                            rz = rzp.tile([128, 4], F32, tag="rz")
                            nc.vector.reciprocal(out=rz[:], in_=o2v[:, :, 0])
                            rzm = rzp.tile([128, 4], F32, tag="rzm")
                            nc.vector.tensor_scalar_mul(
                                out=rzm[:], in0=rz[:], scalar1=vm[:, g:g + 1])
                            if g == 0:
                                nc.vector.tensor_tensor(
                                    out=otv,
                                    in0=o2v[:, :, 1:65],
                                    in1=rzm[:].unsqueeze(2)
                                        .to_broadcast([128, 4, 64]),
                                    op=mybir.AluOpType.mult)
                            else:
                                tmp = outp.tile([128, 256], F32, tag="tmp")
                                nc.vector.tensor_tensor(
                                    out=tmp[:].rearrange(
                                        "p (s d) -> p s d", d=64),
                                    in0=o2v[:, :, 1:65],
                                    in1=rzm[:].unsqueeze(2)
                                        .to_broadcast([128, 4, 64]),
                                    op=mybir.AluOpType.mult)
                                adder = nc.vector if g == 1 else nc.gpsimd
                                adder.tensor_tensor(
                                    out=ot[:], in0=ot[:], in1=tmp[:],
                                    op=mybir.AluOpType.add)
                        nc.sync.dma_start(
                            out=out[bh, p0:p0 + 512, :].rearrange(
                                "(s p) d -> p s d", s=4, p=128),
                            in_=otv)
    nc.compile()
    return nc


def kernel(q, k, v, alpha, _trace=False):
    import ml_dtypes
    from concourse.bass_utils import run_bass_kernel_spmd

    q = np.ascontiguousarray(np.asarray(q, dtype=np.float32))
    k = np.ascontiguousarray(np.asarray(k, dtype=np.float32))
    v = np.ascontiguousarray(np.asarray(v, dtype=np.float32))
    alpha = np.asarray(alpha, dtype=np.float32)

    aw = np.exp(alpha - alpha.max())
    w = aw / aw.sum()

    if "nc" not in _CACHE:
        _CACHE["nc"] = _build()
    nc = _CACHE["nc"]

    mask_k, mask_q = _make_masks(ml_dtypes)
    vmasks = _make_vmasks(w)
    qr = q.reshape(B * H, S, D)
    kr = k.reshape(B * H, S, D)
    vr = v.reshape(B * H, S, D)
    in_maps = []
    for c in range(N_CORES):
        sl = slice(BH_PER_CORE * c, BH_PER_CORE * (c + 1))
        in_maps.append({
            "q": qr[sl], "k": kr[sl], "v": vr[sl],
            "mask_k": mask_k, "mask_q": mask_q, "vmasks": vmasks,
        })
    res = run_bass_kernel_spmd(nc, in_maps, core_ids=list(range(N_CORES)),
                               trace=_trace)
    outs = [res.results[c]["out"] for c in range(N_CORES)]
    full = np.concatenate(outs, axis=0).reshape(B, H, S, D)
    if _trace:
        kernel._last_results = res
    return full



# revision 17
# speedup vs baseline: 5.3807x; 5.3807x over previous
"""Dilated window attention (W=[4,8,16], R=[1,2,4]) on 8 Trainium2 NeuronCores.

Strategy (per core; b*h = 32 shards, 4 per core, grouped in 2 bh-pairs):

  * All three dilation groups share the SAME raw scores: for every 128-position
    strip, S^T[k,q] = K.Q^T is computed ONCE on TensorE (d on partitions,
    bh-pair member m at partition base 64m), and E = exp(S^T/8) ONCE on
    ScalarE (PSUM -> SBUF, bf16).  The per-group masking + softmax
    normalization + group mixing is collapsed into a single rank-56 linear
    form:
        T = U.E           (U = 56 window-membership rows: 32+16+8 windows;
                           rows of T are the per-window softmax denominators
                           Z_g(q) wherever u_g(q)=1)
        Y = (w_g u)/T     (one elementwise divide on DVE; zero off-support)
        F = U^T.Y         (F[k,q] = sum_g w_g M_g[k,q]/Z_g(q), the complete
                           normalized mask-and-mix coefficient)
        A = E * F         (one DVE multiply -> the fully normalized,
                           group-summed attention matrix)
        out = A^T.V       (one AV matmul per strip)
    No per-group mask multiplies, no per-group exp, no per-group AV, no
    output-side normalization or adds.
  * T is partition-packed (q-halves at partition bases 0 and 64) and
    tile-PAIRED, so the divide processes [120, 512] once per 1024 positions;
    AV outputs are likewise paired into one PSUM bank so the PSUM->SBUF
    output copy (ScalarE) runs once per 1024 positions.
  * All layout work is hoisted to the host: Q,K arrive pre-cast to bf16,
    pair-interleaved AND transposed to [d, pos] (plain wide-descriptor DMA,
    no on-device cast pass, no xbar transpose); V arrives bf16 in
    partition-major [128, S/128, 64] layout (2 KiB descriptors); the output
    is written bf16 in the same partition-major layout and unpermuted/upcast
    on the host.
"""
import numpy as np

B, H, S, D = 2, 16, 8192, 64
N_CORES = 8
BH_PER_CORE = (B * H) // N_CORES      # 4 -> 2 bh-pairs
N_PAIRS = BH_PER_CORE // 2
RS = [1, 2, 4]                        # within-window stride per group
WS = [4, 8, 16]                       # window sizes
NWIN = [128 // w for w in WS]         # windows per 128-strip: 32, 16, 8
RTOT = sum(NWIN)                      # 56 stacked membership rows
NCHUNK = S // 128                     # 64 partition-major V/out chunks

_CACHE = {}


def _make_u():
    """[56, 128] 0/1 membership: row (g,c) selects strip-local positions of
    window c that lie on group g's dilation grid."""
    u = np.zeros((RTOT, 128), np.float32)
    r0 = 0
    for g, (w, r) in enumerate(zip(WS, RS)):
        for c in range(128 // w):
            u[r0 + c, w * c:w * (c + 1):r] = 1.0
        r0 += 128 // w
    return u


def _make_consts(w, ml_dtypes):
    u = _make_u()
    uqw = np.zeros_like(u)
    r0 = 0
    for g in range(3):
        uqw[r0:r0 + NWIN[g]] = w[g] * u[r0:r0 + NWIN[g]]
        r0 += NWIN[g]
    bf16 = ml_dtypes.bfloat16
    # T is partition-packed: q-half 0 rows at partitions 0..63, half 1 at
    # 64..127 (membership rows 0..55 each; filler rows 56..63 of ut are 1.0
    # so every T row is written and positive -> reciprocal stays finite).
    # The F matmuls use base-0 full-K weights with the inactive half zeroed
    # (mixed lhsT partition bases into one PSUM bank hang the hardware).
    ut2 = np.ones((128, 64), np.float32)
    ut2[:, 0:RTOT] = u.T
    ufa = np.zeros((128, 128), np.float32)
    ufb = np.zeros((128, 128), np.float32)
    uqw2 = np.zeros((128, 128), np.float32)
    ufa[0:RTOT] = u
    ufb[64:64 + RTOT] = u
    uqw2[0:RTOT] = uqw
    uqw2[64:64 + RTOT] = uqw
    return (ut2.astype(bf16),                         # ut  [128, 64]
            ufa.astype(bf16),                         # ufa [128, 128]
            ufb.astype(bf16),                         # ufb [128, 128]
            uqw2.astype(bf16))                        # uqw2 [128, 128]


def _build(reps=1, qt_bufs=4, v_chunked=True, ymul_eng='vector',
           copy_defer=False, copy_split=False, copy_eng='act', st_bufs=2, t_bufs=2,
           f_bufs=2, o_bufs=2, y_bufs=6, e_bufs=8, a_bufs=8,
           lim_pairs=None, lim_m=None, lim_mt=None):
    import concourse.bacc as bacc
    import concourse.tile as tile
    from concourse import mybir

    F32 = mybir.dt.float32
    BF16 = mybir.dt.bfloat16

    nc = bacc.Bacc("TRN2", target_bir_lowering=False, debug=False,
                   num_devices=N_CORES)
    # [q/k, pair, (member, d), pos] bf16: host-transposed, pair-interleaved
    qk_t = nc.dram_tensor("qk_t", [2, N_PAIRS, 128, S], BF16,
                          kind="ExternalInput")
    # [bh, partition, chunk, d] bf16: host partition-major V
    v_t = nc.dram_tensor("v_t", [BH_PER_CORE, 128, NCHUNK, D], BF16,
                         kind="ExternalInput")
    ut = nc.dram_tensor("ut", [128, 64], BF16, kind="ExternalInput")
    ufa = nc.dram_tensor("ufa", [128, 128], BF16, kind="ExternalInput")
    ufb = nc.dram_tensor("ufb", [128, 128], BF16, kind="ExternalInput")
    uqw2 = nc.dram_tensor("uqw2", [128, 128], BF16, kind="ExternalInput")
    # [bh, partition, chunk, d] bf16: partition-major output
    out = nc.dram_tensor("out", [BH_PER_CORE, 128, NCHUNK, D], BF16,
                         kind="ExternalOutput")

    NT = S // 512                      # 512-position tiles per bh

    with tile.TileContext(nc) as tc:
        with tc.tile_pool(name="const", bufs=1) as constp, \
             tc.tile_pool(name="qt", bufs=qt_bufs) as qtp, \
             tc.tile_pool(name="vaug", bufs=2) as vaugp, \
             tc.tile_pool(name="ep", bufs=e_bufs) as ep, \
             tc.tile_pool(name="yp", bufs=y_bufs) as yp, \
             tc.tile_pool(name="ap", bufs=a_bufs) as apl, \
             tc.tile_pool(name="stage", bufs=3) as stagep, \
             tc.tile_pool(name="stp", bufs=st_bufs, space="PSUM") as stp, \
             tc.tile_pool(name="tp", bufs=t_bufs, space="PSUM") as tp, \
             tc.tile_pool(name="fp", bufs=f_bufs, space="PSUM") as fp, \
             tc.tile_pool(name="op", bufs=o_bufs, space="PSUM") as op:

            utc = constp.tile([128, 64], BF16, tag="utc")
            ufca = constp.tile([128, 128], BF16, tag="ufca")
            ufcb = constp.tile([128, 128], BF16, tag="ufcb")
            uqwc = constp.tile([128, 128], BF16, tag="uqwc")
            nc.sync.dma_start(out=utc[:], in_=ut[:])
            nc.sync.dma_start(out=ufca[:], in_=ufa[:])
            nc.sync.dma_start(out=ufcb[:], in_=ufb[:])
            nc.sync.dma_start(out=uqwc[:], in_=uqw2[:])

            pend_copies = []
            pend_stores = []
            for rep in range(reps):
              for pair in range(lim_pairs or N_PAIRS):
                qt = qtp.tile([128, S], BF16, tag="qt")
                kt = qtp.tile([128, S], BF16, tag="kt")
                nc.sync.dma_start(out=qt[:], in_=qk_t[0, pair])
                nc.sync.dma_start(out=kt[:], in_=qk_t[1, pair])

                for m in range(lim_m or 2):
                    bh = 2 * pair + m
                    if not v_chunked:
                        vaug4 = vaugp.tile([128, NCHUNK, 64], BF16,
                                           tag="vaug")
                        nc.sync.dma_start(out=vaug4[:], in_=v_t[bh])
                        voff = 0
                    for mt in range(lim_mt or NT):
                        p0 = 512 * mt
                        pp = mt % 2          # slot within the tile pair
                        if mt % 4 == 0:
                            if v_chunked:
                                vaug4 = vaugp.tile([128, 16, 64], BF16,
                                                   tag="vaug")
                                nc.sync.dma_start(
                                    out=vaug4[:],
                                    in_=v_t[bh, :, 4 * mt:4 * mt + 16, :])
                                voff = 16 * (mt // 4)
                            stage = stagep.tile([128, 16, 64], BF16,
                                                tag="stage")

                        if copy_defer and pend_copies and pp == 0:
                            pend_copies.pop(0)()
                            if len(pend_stores) and mt % 4 == 2:
                                pend_stores.pop(0)()
                        st = stp.tile([128, 512], F32, tag="st", space="PSUM")
                        for s4 in range(4):
                            c0 = p0 + 128 * s4
                            nc.tensor.matmul(
                                out=st[:, 128 * s4:128 * s4 + 128],
                                lhsT=kt[64 * m:64 * m + 64, c0:c0 + 128],
                                rhs=qt[64 * m:64 * m + 64, c0:c0 + 128],
                                start=True, stop=True)

                        if pp == 0:
                            e2 = ep.tile([128, 2, 512], BF16, tag="e")
                        e = e2[:, pp, :]
                        nc.scalar.activation(
                            out=e, in_=st[:],
                            func=mybir.ActivationFunctionType.Exp,
                            scale=float(D) ** -0.5)

                        # T packed: q-halves at partition bases 0/64,
                        # tile-pair slots along the free dim
                        if pp == 0:
                            t4 = tp.tile([128, 2, 256], F32, tag="t",
                                         space="PSUM")
                        nc.tensor.matmul(
                            out=t4[0:64, pp, :],
                            lhsT=utc[:], rhs=e[:, 0:256],
                            start=True, stop=True)
                        nc.tensor.matmul(
                            out=t4[64:128, pp, :],
                            lhsT=utc[:], rhs=e[:, 256:512],
                            start=True, stop=True)
                        del e

                        if pp == 1:
                            r = yp.tile([128, 2, 256], BF16, tag="r")
                            with nc.allow_low_precision(reason="1/Z bf16"):
                                nc.vector.reciprocal(out=r[:], in_=t4[:])
                            y = yp.tile([128, 2, 256], BF16, tag="y")
                            ymul = (nc.gpsimd if ymul_eng == 'gpsimd'
                                    else nc.vector)
                            ymul.tensor_tensor(
                                out=y[:].rearrange(
                                    "p a (b c) -> p a b c", c=128),
                                in0=uqwc[:].unsqueeze(1).unsqueeze(1)
                                    .to_broadcast([128, 2, 2, 128]),
                                in1=r[:].rearrange(
                                    "p a (b c) -> p a b c", c=128),
                                op=mybir.AluOpType.mult)

                            o2 = op.tile([128, 2, 4, 64], F32, tag="o2",
                                         space="PSUM")
                            f2 = fp.tile([128, 2, 512], F32, tag="f",
                                         space="PSUM")
                            for qq in range(2):
                                nc.tensor.matmul(
                                    out=f2[:, qq, 0:256],
                                    lhsT=ufca[:],
                                    rhs=y[:, qq, :],
                                    start=True, stop=True)
                                nc.tensor.matmul(
                                    out=f2[:, qq, 256:512],
                                    lhsT=ufcb[:],
                                    rhs=y[:, qq, :],
                                    start=True, stop=True)
                            a2 = apl.tile([128, 2, 512], BF16, tag="a")
                            nc.vector.tensor_tensor(
                                out=a2[:], in0=e2[:], in1=f2[:],
                                op=mybir.AluOpType.mult)
                            for qq in range(2):
                                vb = (4 * ((mt - 1 + qq) % 4) if v_chunked
                                      else 4 * (mt - 1 + qq))
                                for s4 in range(4):
                                    nc.tensor.matmul(
                                        out=o2[:, qq, s4, :],
                                        lhsT=a2[:, qq,
                                                128 * s4:128 * s4 + 128],
                                        rhs=vaug4[:, vb + s4, :],
                                        start=True, stop=True)
                            so = 8 * (((mt - 1) % 4) // 2)
                            ostage = stage
                            def emit_copy(o2=o2, so=so, ostage=ostage):
                                dst = ostage[:, so:so + 8, :].rearrange(
                                    "p (a s) d -> p a s d", a=2)
                                ceng = copy_eng
                                if ceng == 'alt':
                                    ceng = 'act' if (mt // 2) % 2 == 0 \
                                        else 'dve'
                                if copy_split:
                                    nc.scalar.copy(out=dst[:, 0], in_=o2[:, 0])
                                    nc.scalar.copy(out=dst[:, 1], in_=o2[:, 1])
                                elif ceng == 'act':
                                    nc.scalar.copy(out=dst, in_=o2[:])
                                else:
                                    nc.vector.tensor_copy(out=dst, in_=o2[:])
                            if copy_defer:
                                pend_copies.append(emit_copy)
                            else:
                                emit_copy()

                        if mt % 4 == 3:
                            ostage = stage
                            def emit_store(bh=bh, mt=mt, ostage=ostage):
                                nc.sync.dma_start(
                                    out=out[bh, :, 4 * mt - 12:4 * mt + 4, :],
                                    in_=ostage[:])
                            if copy_defer:
                                pend_stores.append(emit_store)
                            else:
                                emit_store()
            for fn in pend_copies:
                fn()
            for fn in pend_stores:
                fn()
    nc.compile()
    return nc


def kernel(q, k, v, alpha, _trace=False):
    import ml_dtypes
    from concourse.bass_utils import run_bass_kernel_spmd

    q = np.ascontiguousarray(np.asarray(q, dtype=np.float32))
    k = np.ascontiguousarray(np.asarray(k, dtype=np.float32))
    v = np.ascontiguousarray(np.asarray(v, dtype=np.float32))
    alpha = np.asarray(alpha, dtype=np.float32)

    aw = np.exp(alpha - alpha.max())
    w = aw / aw.sum()

    if "nc" not in _CACHE:
        _CACHE["nc"] = _build()
    nc = _CACHE["nc"]

    ut, ufa, ufb, uqw2 = _make_consts(w, ml_dtypes)
    bf16 = ml_dtypes.bfloat16
    # [2, pairs, 128=(m,d), S]: pre-cast bf16, pair-interleaved, transposed
    qr = q.reshape(B * H // 2, 2, S, D)
    kr = k.reshape(B * H // 2, 2, S, D)
    qk = np.stack([qr, kr], axis=0).transpose(0, 1, 2, 4, 3).reshape(
        2, B * H // 2, 128, S).astype(bf16)
    # [bh, 128, chunk, d]: partition-major V
    vt = v.reshape(B * H, NCHUNK, 128, D).transpose(0, 2, 1, 3).astype(bf16)
    in_maps = []
    for c in range(N_CORES):
        in_maps.append({
            "qk_t": np.ascontiguousarray(
                qk[:, N_PAIRS * c:N_PAIRS * (c + 1)]),
            "v_t": np.ascontiguousarray(
                vt[BH_PER_CORE * c:BH_PER_CORE * (c + 1)]),
            "ut": ut, "ufa": ufa, "ufb": ufb, "uqw2": uqw2,
        })
    res = run_bass_kernel_spmd(nc, in_maps, core_ids=list(range(N_CORES)),
                               trace=_trace)
    outs = [res.results[c]["out"] for c in range(N_CORES)]
    # [BH, 128, chunk, d] bf16 -> [B, H, S, D] f32
    full = np.concatenate(outs, axis=0).astype(np.float32)
    full = full.transpose(0, 2, 1, 3).reshape(B, H, S, D)
    if _trace:
        kernel._last_results = res
    return full


# revision 22
# speedup vs baseline: 6.0930x; 1.1324x over previous
"""Dilated window attention (W=[4,8,16], R=[1,2,4]) on 8 Trainium2 NeuronCores.

Strategy (per core; b*h = 32 shards, 4 per core, grouped in 2 bh-pairs):

  * All three dilation groups share the SAME raw scores: for every 128-position
    strip, S^T[k,q] = K.Q^T is computed ONCE on TensorE (d on partitions,
    bh-pair member m at partition base 64m), and E = exp(S^T/8) ONCE on
    ScalarE (PSUM -> SBUF, bf16).  The per-group masking + softmax
    normalization + group mixing is collapsed into a single rank-56 linear
    form:
        T = U.E           (U = 56 window-membership rows: 32+16+8 windows;
                           rows of T are the per-window softmax denominators
                           Z_g(q) wherever u_g(q)=1)
        Y = (w_g u)/T     (one elementwise divide on DVE; zero off-support)
        F = U^T.Y         (F[k,q] = sum_g w_g M_g[k,q]/Z_g(q), the complete
                           normalized mask-and-mix coefficient)
        A = E * F         (one DVE multiply -> the fully normalized,
                           group-summed attention matrix)
        out = A^T.V       (one AV matmul per strip)
    No per-group mask multiplies, no per-group exp, no per-group AV, no
    output-side normalization or adds.
  * T is partition-packed (q-halves at partition bases 0 and 64) and
    tile-PAIRED, so the divide processes [120, 512] once per 1024 positions;
    AV outputs are likewise paired into one PSUM bank so the PSUM->SBUF
    output copy (ScalarE) runs once per 1024 positions.
  * All layout work is hoisted to the host: Q,K arrive pre-cast to bf16,
    pair-interleaved AND transposed to [d, pos] (plain wide-descriptor DMA,
    no on-device cast pass, no xbar transpose); V arrives bf16 in
    partition-major [128, S/128, 64] layout (2 KiB descriptors); the output
    is written bf16 in the same partition-major layout and unpermuted/upcast
    on the host.
"""
import numpy as np

B, H, S, D = 2, 16, 8192, 64
N_CORES = 8
BH_PER_CORE = (B * H) // N_CORES      # 4 -> 2 bh-pairs
N_PAIRS = BH_PER_CORE // 2
RS = [1, 2, 4]                        # within-window stride per group
WS = [4, 8, 16]                       # window sizes
NWIN = [128 // w for w in WS]         # windows per 128-strip: 32, 16, 8
RTOT = sum(NWIN)                      # 56 stacked membership rows
NCHUNK = S // 128                     # 64 partition-major V/out chunks

_CACHE = {}


def _make_u():
    """[56, 128] 0/1 membership: row (g,c) selects strip-local positions of
    window c that lie on group g's dilation grid."""
    u = np.zeros((RTOT, 128), np.float32)
    r0 = 0
    for g, (w, r) in enumerate(zip(WS, RS)):
        for c in range(128 // w):
            u[r0 + c, w * c:w * (c + 1):r] = 1.0
        r0 += 128 // w
    return u


def _make_consts(w, ml_dtypes):
    u = _make_u()
    uqw = np.zeros_like(u)
    r0 = 0
    for g in range(3):
        uqw[r0:r0 + NWIN[g]] = w[g] * u[r0:r0 + NWIN[g]]
        r0 += NWIN[g]
    bf16 = ml_dtypes.bfloat16
    # T is partition-packed: q-half 0 rows at partitions 0..63, half 1 at
    # 64..127 (membership rows 0..55 each; filler rows 56..63 of ut are 1.0
    # so every T row is written and positive -> reciprocal stays finite).
    # The F matmuls use base-0 full-K weights with the inactive half zeroed
    # (mixed lhsT partition bases into one PSUM bank hang the hardware).
    ut2 = np.ones((128, 64), np.float32)
    ut2[:, 0:RTOT] = u.T
    ufa = np.zeros((128, 128), np.float32)
    ufb = np.zeros((128, 128), np.float32)
    uqw2 = np.zeros((128, 128), np.float32)
    ufa[0:RTOT] = u
    ufb[64:64 + RTOT] = u
    uqw2[0:RTOT] = uqw
    uqw2[64:64 + RTOT] = uqw
    return (ut2.astype(bf16),                         # ut  [128, 64]
            ufa.astype(bf16),                         # ufa [128, 128]
            ufb.astype(bf16),                         # ufb [128, 128]
            uqw2.astype(bf16))                        # uqw2 [128, 128]


def _build(reps=1, qt_bufs=4, v_chunked=True, ymul_eng='gpsimd',
           copy_defer=False, copy_split=False, copy_eng='act', st_bufs=2, t_bufs=2,
           f_bufs=1, o_bufs=2, y_bufs=6, e_bufs=8, a_bufs=8,
           exp_paired=False, ef_late=0, out_late=0,
           lim_pairs=None, lim_m=None, lim_mt=None):
    import concourse.bacc as bacc
    import concourse.tile as tile
    from concourse import mybir

    F32 = mybir.dt.float32
    BF16 = mybir.dt.bfloat16

    nc = bacc.Bacc("TRN2", target_bir_lowering=False, debug=False,
                   num_devices=N_CORES)
    # [q/k, pair, (member, d), pos] bf16: host-transposed, pair-interleaved
    qk_t = nc.dram_tensor("qk_t", [2, N_PAIRS, 128, S], BF16,
                          kind="ExternalInput")
    # [bh, partition, chunk, d] bf16: host partition-major V
    v_t = nc.dram_tensor("v_t", [BH_PER_CORE, 128, NCHUNK, D], BF16,
                         kind="ExternalInput")
    ut = nc.dram_tensor("ut", [128, 64], BF16, kind="ExternalInput")
    ufa = nc.dram_tensor("ufa", [128, 128], BF16, kind="ExternalInput")
    ufb = nc.dram_tensor("ufb", [128, 128], BF16, kind="ExternalInput")
    uqw2 = nc.dram_tensor("uqw2", [128, 128], BF16, kind="ExternalInput")
    # [bh, partition, chunk, d] bf16: partition-major output
    out = nc.dram_tensor("out", [BH_PER_CORE, 128, NCHUNK, D], BF16,
                         kind="ExternalOutput")

    NT = S // 512                      # 512-position tiles per bh

    with tile.TileContext(nc) as tc:
        with tc.tile_pool(name="const", bufs=1) as constp, \
             tc.tile_pool(name="qt", bufs=qt_bufs) as qtp, \
             tc.tile_pool(name="vaug", bufs=2) as vaugp, \
             tc.tile_pool(name="ep", bufs=e_bufs) as ep, \
             tc.tile_pool(name="yp", bufs=y_bufs) as yp, \
             tc.tile_pool(name="ap", bufs=a_bufs) as apl, \
             tc.tile_pool(name="stage", bufs=3) as stagep, \
             tc.tile_pool(name="stp", bufs=st_bufs, space="PSUM") as stp, \
             tc.tile_pool(name="tp", bufs=t_bufs, space="PSUM") as tp, \
             tc.tile_pool(name="fp", bufs=f_bufs, space="PSUM") as fp, \
             tc.tile_pool(name="op", bufs=o_bufs, space="PSUM") as op:

            utc = constp.tile([128, 64], BF16, tag="utc")
            ufca = constp.tile([128, 128], BF16, tag="ufca")
            ufcb = constp.tile([128, 128], BF16, tag="ufcb")
            uqwc = constp.tile([128, 128], BF16, tag="uqwc")
            nc.sync.dma_start(out=utc[:], in_=ut[:])
            nc.sync.dma_start(out=ufca[:], in_=ufa[:])
            nc.sync.dma_start(out=ufcb[:], in_=ufb[:])
            nc.sync.dma_start(out=uqwc[:], in_=uqw2[:])

            pend_copies = []
            pend_stores = []
            for rep in range(reps):
              for pair in range(lim_pairs or N_PAIRS):
                qt = qtp.tile([128, S], BF16, tag="qt")
                kt = qtp.tile([128, S], BF16, tag="kt")
                nc.sync.dma_start(out=qt[:], in_=qk_t[0, pair])
                nc.sync.dma_start(out=kt[:], in_=qk_t[1, pair])

                for m in range(lim_m or 2):
                    bh = 2 * pair + m
                    if not v_chunked:
                        vaug4 = vaugp.tile([128, NCHUNK, 64], BF16,
                                           tag="vaug")
                        nc.sync.dma_start(out=vaug4[:], in_=v_t[bh])
                        voff = 0
                    for mt in range(lim_mt or NT):
                        p0 = 512 * mt
                        pp = mt % 2          # slot within the tile pair
                        if mt % 4 == 0:
                            if v_chunked:
                                vaug4 = vaugp.tile([128, 16, 64], BF16,
                                                   tag="vaug")
                                nc.sync.dma_start(
                                    out=vaug4[:],
                                    in_=v_t[bh, :, 4 * mt:4 * mt + 16, :])
                                voff = 16 * (mt // 4)
                            stage = stagep.tile([128, 16, 64], BF16,
                                                tag="stage")

                        if copy_defer and pend_copies and pp == 0:
                            pend_copies.pop(0)()
                            if len(pend_stores) and mt % 4 == 2:
                                pend_stores.pop(0)()
                        if exp_paired:
                            if pp == 0:
                                st2 = stp.tile([128, 2, 512], F32, tag="st",
                                               space="PSUM")
                            st = st2[:, pp, :]
                        else:
                            st_t = stp.tile([128, 512], F32, tag="st",
                                            space="PSUM")
                            st = st_t[:]
                        for s4 in range(4):
                            c0 = p0 + 128 * s4
                            nc.tensor.matmul(
                                out=st[:, 128 * s4:128 * s4 + 128],
                                lhsT=kt[64 * m:64 * m + 64, c0:c0 + 128],
                                rhs=qt[64 * m:64 * m + 64, c0:c0 + 128],
                                start=True, stop=True)

                        if pp == 0:
                            e2 = ep.tile([128, 2, 512], BF16, tag="e")
                        if exp_paired:
                            if pp == 1:
                                nc.scalar.activation(
                                    out=e2[:], in_=st2[:],
                                    func=mybir.ActivationFunctionType.Exp,
                                    scale=float(D) ** -0.5)
                        else:
                            nc.scalar.activation(
                                out=e2[:, pp, :], in_=st[:],
                                func=mybir.ActivationFunctionType.Exp,
                                scale=float(D) ** -0.5)
                        e = e2[:, pp, :]

                        # T packed: q-halves at partition bases 0/64,
                        # tile-pair slots along the free dim
                        if pp == 0:
                            t4 = tp.tile([128, 2, 256], F32, tag="t",
                                         space="PSUM")
                        nc.tensor.matmul(
                            out=t4[0:64, pp, :],
                            lhsT=utc[:], rhs=e[:, 0:256],
                            start=True, stop=True)
                        nc.tensor.matmul(
                            out=t4[64:128, pp, :],
                            lhsT=utc[:], rhs=e[:, 256:512],
                            start=True, stop=True)
                        del e

                        if pp == 1:
                            r = yp.tile([128, 2, 256], BF16, tag="r")
                            with nc.allow_low_precision(reason="1/Z bf16"):
                                nc.vector.reciprocal(out=r[:], in_=t4[:])
                            y = yp.tile([128, 2, 256], BF16, tag="y")
                            ymul = (nc.gpsimd if ymul_eng == 'gpsimd'
                                    else nc.vector)
                            ymul.tensor_tensor(
                                out=y[:].rearrange(
                                    "p a (b c) -> p a b c", c=128),
                                in0=uqwc[:].unsqueeze(1).unsqueeze(1)
                                    .to_broadcast([128, 2, 2, 128]),
                                in1=r[:].rearrange(
                                    "p a (b c) -> p a b c", c=128),
                                op=mybir.AluOpType.mult)

                            o2 = op.tile([128, 2, 4, 64], F32, tag="o2",
                                         space="PSUM")
                            f2 = fp.tile([128, 2, 512], F32, tag="f",
                                         space="PSUM")
                            for qq in range(2):
                                nc.tensor.matmul(
                                    out=f2[:, qq, 0:256],
                                    lhsT=ufca[:],
                                    rhs=y[:, qq, :],
                                    start=True, stop=True)
                                nc.tensor.matmul(
                                    out=f2[:, qq, 256:512],
                                    lhsT=ufcb[:],
                                    rhs=y[:, qq, :],
                                    start=True, stop=True)
                            a2 = apl.tile([128, 2, 512], BF16, tag="a")
                            with tc.high_priority(offset=-ef_late):
                                nc.vector.tensor_tensor(
                                    out=a2[:], in0=e2[:], in1=f2[:],
                                    op=mybir.AluOpType.mult)
                            for qq in range(2):
                                vb = (4 * ((mt - 1 + qq) % 4) if v_chunked
                                      else 4 * (mt - 1 + qq))
                                for s4 in range(4):
                                    nc.tensor.matmul(
                                        out=o2[:, qq, s4, :],
                                        lhsT=a2[:, qq,
                                                128 * s4:128 * s4 + 128],
                                        rhs=vaug4[:, vb + s4, :],
                                        start=True, stop=True)
                            so = 8 * (((mt - 1) % 4) // 2)
                            ostage = stage
                            tc.cur_priority += out_late
                            def emit_copy(o2=o2, so=so, ostage=ostage):
                                dst = ostage[:, so:so + 8, :].rearrange(
                                    "p (a s) d -> p a s d", a=2)
                                ceng = copy_eng
                                if ceng == 'alt':
                                    ceng = 'act' if (mt // 2) % 2 == 0 \
                                        else 'dve'
                                if ceng == 'split2':
                                    nc.scalar.copy(out=dst[:, 0], in_=o2[:, 0])
                                    nc.vector.tensor_copy(out=dst[:, 1],
                                                          in_=o2[:, 1])
                                elif copy_split:
                                    nc.scalar.copy(out=dst[:, 0], in_=o2[:, 0])
                                    nc.scalar.copy(out=dst[:, 1], in_=o2[:, 1])
                                elif ceng == 'act':
                                    nc.scalar.copy(out=dst, in_=o2[:])
                                else:
                                    nc.vector.tensor_copy(out=dst, in_=o2[:])
                            if copy_defer:
                                pend_copies.append(emit_copy)
                            else:
                                emit_copy()

                        if mt % 4 == 3:
                            ostage = stage
                            def emit_store(bh=bh, mt=mt, ostage=ostage):
                                nc.sync.dma_start(
                                    out=out[bh, :, 4 * mt - 12:4 * mt + 4, :],
                                    in_=ostage[:])
                            if copy_defer:
                                pend_stores.append(emit_store)
                            else:
                                emit_store()
            for fn in pend_copies:
                fn()
            for fn in pend_stores:
                fn()
    nc.compile()
    return nc


def kernel(q, k, v, alpha, _trace=False):
    import ml_dtypes
    from concourse.bass_utils import run_bass_kernel_spmd

    q = np.ascontiguousarray(np.asarray(q, dtype=np.float32))
    k = np.ascontiguousarray(np.asarray(k, dtype=np.float32))
    v = np.ascontiguousarray(np.asarray(v, dtype=np.float32))
    alpha = np.asarray(alpha, dtype=np.float32)

    aw = np.exp(alpha - alpha.max())
    w = aw / aw.sum()

    if "nc" not in _CACHE:
        _CACHE["nc"] = _build()
    nc = _CACHE["nc"]

    ut, ufa, ufb, uqw2 = _make_consts(w, ml_dtypes)
    bf16 = ml_dtypes.bfloat16
    # [2, pairs, 128=(m,d), S]: pre-cast bf16, pair-interleaved, transposed
    qr = q.reshape(B * H // 2, 2, S, D)
    kr = k.reshape(B * H // 2, 2, S, D)
    qk = np.stack([qr, kr], axis=0).transpose(0, 1, 2, 4, 3).reshape(
        2, B * H // 2, 128, S).astype(bf16)
    # [bh, 128, chunk, d]: partition-major V
    vt = v.reshape(B * H, NCHUNK, 128, D).transpose(0, 2, 1, 3).astype(bf16)
    in_maps = []
    for c in range(N_CORES):
        in_maps.append({
            "qk_t": np.ascontiguousarray(
                qk[:, N_PAIRS * c:N_PAIRS * (c + 1)]),
            "v_t": np.ascontiguousarray(
                vt[BH_PER_CORE * c:BH_PER_CORE * (c + 1)]),
            "ut": ut, "ufa": ufa, "ufb": ufb, "uqw2": uqw2,
        })
    res = run_bass_kernel_spmd(nc, in_maps, core_ids=list(range(N_CORES)),
                               trace=_trace)
    outs = [res.results[c]["out"] for c in range(N_CORES)]
    # [BH, 128, chunk, d] bf16 -> [B, H, S, D] f32
    full = np.concatenate(outs, axis=0).astype(np.float32)
    full = full.transpose(0, 2, 1, 3).reshape(B, H, S, D)
    if _trace:
        kernel._last_results = res
    return full


# revision 25
# speedup vs baseline: 6.2448x; 1.0249x over previous
"""Dilated window attention (W=[4,8,16], R=[1,2,4]) on 8 Trainium2 NeuronCores.

Strategy (per core; b*h = 32 shards, 4 per core, grouped in 2 bh-pairs):

  * All three dilation groups share the SAME raw scores: for every 128-position
    strip, S^T[k,q] = K.Q^T is computed ONCE on TensorE (d on partitions,
    bh-pair member m at partition base 64m), and E = exp(S^T/8) ONCE on
    ScalarE (PSUM -> SBUF, bf16).  The per-group masking + softmax
    normalization + group mixing is collapsed into a single rank-56 linear
    form:
        T = U.E            (U = 56 window-membership rows: 32+16+8 windows;
                            rows of T are the per-window softmax denominators
                            Z_g(q) wherever u_g(q)=1)
        R = 1/T            (DVE reciprocal, bf16)
        Y = (w_g u) * R    (GpSimd multiply; zero off-support)
        F = U^T.Y          (F[k,q] = sum_g w_g M_g[k,q]/Z_g(q), the complete
                            normalized mask-and-mix coefficient)
        A = E * F          (one DVE multiply -> the fully normalized,
                            group-summed attention matrix)
        out = A^T.V        (one AV matmul per strip)
    No per-group mask multiplies, no per-group exp, no per-group AV, no
    output-side normalization or adds.
  * T is partition-packed (q-halves at partitions 0-63 / 64-127, with 1.0
    filler columns in U so every T row is written and positive -> finite
    reciprocal) and tile-PAIRED, so R/Y process [128, 512] once per 1024
    positions; E/F/A are pair-tiles so the A = E*F multiply runs once per
    1024 positions; AV outputs share one PSUM bank per pair so the
    PSUM->SBUF output copy (ScalarE) also runs once per pair.  The two F
    matmuls use base-0 full-K weights with the inactive half zeroed
    (back-to-back matmuls with different lhsT partition bases into one PSUM
    bank hang the hardware).
  * All layout work is hoisted to the host: Q,K arrive pre-cast to bf16,
    pair-interleaved AND transposed to [d, pos] (plain wide-descriptor DMAs,
    no on-device cast pass, no xbar transpose); V arrives bf16 in
    partition-major [128, S/128, 64] layout (2 KiB descriptors); the output
    is written bf16 in the same partition-major layout and unpermuted/upcast
    on the host.
"""
import numpy as np

B, H, S, D = 2, 16, 8192, 64
N_CORES = 8
BH_PER_CORE = (B * H) // N_CORES      # 4 -> 2 bh-pairs
N_PAIRS = BH_PER_CORE // 2
RS = [1, 2, 4]                        # within-window stride per group
WS = [4, 8, 16]                       # window sizes
NWIN = [128 // w for w in WS]         # windows per 128-strip: 32, 16, 8
RTOT = sum(NWIN)                      # 56 stacked membership rows
NCHUNK = S // 128                     # 64 partition-major V/out chunks

_CACHE = {}


def _make_u():
    """[56, 128] 0/1 membership: row (g,c) selects strip-local positions of
    window c that lie on group g's dilation grid."""
    u = np.zeros((RTOT, 128), np.float32)
    r0 = 0
    for g, (w, r) in enumerate(zip(WS, RS)):
        for c in range(128 // w):
            u[r0 + c, w * c:w * (c + 1):r] = 1.0
        r0 += 128 // w
    return u


def _make_consts(w, ml_dtypes):
    u = _make_u()
    uqw = np.zeros_like(u)
    r0 = 0
    for g in range(3):
        uqw[r0:r0 + NWIN[g]] = w[g] * u[r0:r0 + NWIN[g]]
        r0 += NWIN[g]
    bf16 = ml_dtypes.bfloat16
    # T is partition-packed: q-half 0 rows at partitions 0..63, half 1 at
    # 64..127 (membership rows 0..55 each; filler rows 56..63 of ut are 1.0
    # so every T row is written and positive -> reciprocal stays finite).
    # The F matmuls use base-0 full-K weights with the inactive half zeroed
    # (mixed lhsT partition bases into one PSUM bank hang the hardware).
    ut2 = np.ones((128, 64), np.float32)
    ut2[:, 0:RTOT] = u.T
    ufa = np.zeros((128, 128), np.float32)
    ufb = np.zeros((128, 128), np.float32)
    uqw2 = np.zeros((128, 128), np.float32)
    ufa[0:RTOT] = u
    ufb[64:64 + RTOT] = u
    uqw2[0:RTOT] = uqw
    uqw2[64:64 + RTOT] = uqw
    return (ut2.astype(bf16),                         # ut  [128, 64]
            ufa.astype(bf16),                         # ufa [128, 128]
            ufb.astype(bf16),                         # ufb [128, 128]
            uqw2.astype(bf16))                        # uqw2 [128, 128]


def _build(reps=1, qt_bufs=4, v_chunked=True, ymul_eng='gpsimd',
           copy_defer=False, copy_split=False, copy_eng='act', st_bufs=2, t_bufs=2,
           f_bufs=1, o_bufs=1, y_bufs=6, e_bufs=8, a_bufs=8,
           exp_paired=False, ef_late=0, out_late=0, qk_chunk=1024,
           store_pair=False,
           lim_pairs=None, lim_m=None, lim_mt=None):
    import concourse.bacc as bacc
    import concourse.tile as tile
    from concourse import mybir

    F32 = mybir.dt.float32
    BF16 = mybir.dt.bfloat16

    nc = bacc.Bacc("TRN2", target_bir_lowering=False, debug=False,
                   num_devices=N_CORES)
    # [q/k, pair, (member, d), pos] bf16: host-transposed, pair-interleaved
    qk_t = nc.dram_tensor("qk_t", [2, N_PAIRS, 128, S], BF16,
                          kind="ExternalInput")
    # [bh, partition, chunk, d] bf16: host partition-major V
    v_t = nc.dram_tensor("v_t", [BH_PER_CORE, 128, NCHUNK, D], BF16,
                         kind="ExternalInput")
    ut = nc.dram_tensor("ut", [128, 64], BF16, kind="ExternalInput")
    ufa = nc.dram_tensor("ufa", [128, 128], BF16, kind="ExternalInput")
    ufb = nc.dram_tensor("ufb", [128, 128], BF16, kind="ExternalInput")
    uqw2 = nc.dram_tensor("uqw2", [128, 128], BF16, kind="ExternalInput")
    # [bh, partition, chunk, d] bf16: partition-major output
    out = nc.dram_tensor("out", [BH_PER_CORE, 128, NCHUNK, D], BF16,
                         kind="ExternalOutput")

    NT = S // 512                      # 512-position tiles per bh

    with tile.TileContext(nc) as tc:
        with tc.tile_pool(name="const", bufs=1) as constp, \
             tc.tile_pool(name="qt", bufs=qt_bufs) as qtp, \
             tc.tile_pool(name="vaug", bufs=2) as vaugp, \
             tc.tile_pool(name="ep", bufs=e_bufs) as ep, \
             tc.tile_pool(name="yp", bufs=y_bufs) as yp, \
             tc.tile_pool(name="ap", bufs=a_bufs) as apl, \
             tc.tile_pool(name="stage", bufs=3) as stagep, \
             tc.tile_pool(name="stp", bufs=st_bufs, space="PSUM") as stp, \
             tc.tile_pool(name="tp", bufs=t_bufs, space="PSUM") as tp, \
             tc.tile_pool(name="fp", bufs=f_bufs, space="PSUM") as fp, \
             tc.tile_pool(name="op", bufs=o_bufs, space="PSUM") as op:

            utc = constp.tile([128, 64], BF16, tag="utc")
            ufca = constp.tile([128, 128], BF16, tag="ufca")
            ufcb = constp.tile([128, 128], BF16, tag="ufcb")
            uqwc = constp.tile([128, 128], BF16, tag="uqwc")
            nc.sync.dma_start(out=utc[:], in_=ut[:])
            nc.sync.dma_start(out=ufca[:], in_=ufa[:])
            nc.sync.dma_start(out=ufcb[:], in_=ufb[:])
            nc.sync.dma_start(out=uqwc[:], in_=uqw2[:])

            pend_copies = []
            pend_stores = []
            for rep in range(reps):
              for pair in range(lim_pairs or N_PAIRS):
                qt = qtp.tile([128, S], BF16, tag="qt")
                kt = qtp.tile([128, S], BF16, tag="kt")
                if qk_chunk:
                    for c0 in range(0, S, qk_chunk):
                        nc.sync.dma_start(
                            out=qt[:, c0:c0 + qk_chunk],
                            in_=qk_t[0, pair, :, c0:c0 + qk_chunk])
                        nc.sync.dma_start(
                            out=kt[:, c0:c0 + qk_chunk],
                            in_=qk_t[1, pair, :, c0:c0 + qk_chunk])
                else:
                    nc.sync.dma_start(out=qt[:], in_=qk_t[0, pair])
                    nc.sync.dma_start(out=kt[:], in_=qk_t[1, pair])

                for m in range(lim_m or 2):
                    bh = 2 * pair + m
                    if not v_chunked:
                        vaug4 = vaugp.tile([128, NCHUNK, 64], BF16,
                                           tag="vaug")
                        nc.sync.dma_start(out=vaug4[:], in_=v_t[bh])
                        voff = 0
                    for mt in range(lim_mt or NT):
                        p0 = 512 * mt
                        pp = mt % 2          # slot within the tile pair
                        if mt % 4 == 0:
                            if v_chunked:
                                vaug4 = vaugp.tile([128, 16, 64], BF16,
                                                   tag="vaug")
                                nc.sync.dma_start(
                                    out=vaug4[:],
                                    in_=v_t[bh, :, 4 * mt:4 * mt + 16, :])
                                voff = 16 * (mt // 4)
                            if not store_pair:
                                stage = stagep.tile([128, 16, 64], BF16,
                                                    tag="stage")

                        if copy_defer and pend_copies and pp == 0:
                            pend_copies.pop(0)()
                            if len(pend_stores) and mt % 4 == 2:
                                pend_stores.pop(0)()
                        if exp_paired:
                            if pp == 0:
                                st2 = stp.tile([128, 2, 512], F32, tag="st",
                                               space="PSUM")
                            st = st2[:, pp, :]
                        else:
                            st_t = stp.tile([128, 512], F32, tag="st",
                                            space="PSUM")
                            st = st_t[:]
                        for s4 in range(4):
                            c0 = p0 + 128 * s4
                            nc.tensor.matmul(
                                out=st[:, 128 * s4:128 * s4 + 128],
                                lhsT=kt[64 * m:64 * m + 64, c0:c0 + 128],
                                rhs=qt[64 * m:64 * m + 64, c0:c0 + 128],
                                start=True, stop=True)

                        if pp == 0:
                            e2 = ep.tile([128, 2, 512], BF16, tag="e")
                        if exp_paired:
                            if pp == 1:
                                nc.scalar.activation(
                                    out=e2[:], in_=st2[:],
                                    func=mybir.ActivationFunctionType.Exp,
                                    scale=float(D) ** -0.5)
                        else:
                            nc.scalar.activation(
                                out=e2[:, pp, :], in_=st[:],
                                func=mybir.ActivationFunctionType.Exp,
                                scale=float(D) ** -0.5)
                        e = e2[:, pp, :]

                        # T packed: q-halves at partition bases 0/64,
                        # tile-pair slots along the free dim
                        if pp == 0:
                            t4 = tp.tile([128, 2, 256], F32, tag="t",
                                         space="PSUM")
                        nc.tensor.matmul(
                            out=t4[0:64, pp, :],
                            lhsT=utc[:], rhs=e[:, 0:256],
                            start=True, stop=True)
                        nc.tensor.matmul(
                            out=t4[64:128, pp, :],
                            lhsT=utc[:], rhs=e[:, 256:512],
                            start=True, stop=True)
                        del e

                        if pp == 1:
                            r = yp.tile([128, 2, 256], BF16, tag="r")
                            with nc.allow_low_precision(reason="1/Z bf16"):
                                nc.vector.reciprocal(out=r[:], in_=t4[:])
                            y = yp.tile([128, 2, 256], BF16, tag="y")
                            ymul = (nc.gpsimd if ymul_eng == 'gpsimd'
                                    else nc.vector)
                            ymul.tensor_tensor(
                                out=y[:].rearrange(
                                    "p a (b c) -> p a b c", c=128),
                                in0=uqwc[:].unsqueeze(1).unsqueeze(1)
                                    .to_broadcast([128, 2, 2, 128]),
                                in1=r[:].rearrange(
                                    "p a (b c) -> p a b c", c=128),
                                op=mybir.AluOpType.mult)

                            o2 = op.tile([128, 2, 4, 64], F32, tag="o2",
                                         space="PSUM")
                            f2 = fp.tile([128, 2, 512], F32, tag="f",
                                         space="PSUM")
                            for qq in range(2):
                                nc.tensor.matmul(
                                    out=f2[:, qq, 0:256],
                                    lhsT=ufca[:],
                                    rhs=y[:, qq, :],
                                    start=True, stop=True)
                                nc.tensor.matmul(
                                    out=f2[:, qq, 256:512],
                                    lhsT=ufcb[:],
                                    rhs=y[:, qq, :],
                                    start=True, stop=True)
                            a2 = apl.tile([128, 2, 512], BF16, tag="a")
                            with tc.high_priority(offset=-ef_late):
                                nc.vector.tensor_tensor(
                                    out=a2[:], in0=e2[:], in1=f2[:],
                                    op=mybir.AluOpType.mult)
                            for qq in range(2):
                                vb = (4 * ((mt - 1 + qq) % 4) if v_chunked
                                      else 4 * (mt - 1 + qq))
                                for s4 in range(4):
                                    nc.tensor.matmul(
                                        out=o2[:, qq, s4, :],
                                        lhsT=a2[:, qq,
                                                128 * s4:128 * s4 + 128],
                                        rhs=vaug4[:, vb + s4, :],
                                        start=True, stop=True)
                            if store_pair:
                                stage = stagep.tile([128, 8, 64], BF16,
                                                    tag="stage")
                                so = 0
                            else:
                                so = 8 * (((mt - 1) % 4) // 2)
                            ostage = stage
                            tc.cur_priority += out_late
                            def emit_copy(o2=o2, so=so, ostage=ostage):
                                dst = ostage[:, so:so + 8, :].rearrange(
                                    "p (a s) d -> p a s d", a=2)
                                ceng = copy_eng
                                if ceng == 'alt':
                                    ceng = 'act' if (mt // 2) % 2 == 0 \
                                        else 'dve'
                                if ceng == 'split2':
                                    nc.scalar.copy(out=dst[:, 0], in_=o2[:, 0])
                                    nc.vector.tensor_copy(out=dst[:, 1],
                                                          in_=o2[:, 1])
                                elif copy_split:
                                    nc.scalar.copy(out=dst[:, 0], in_=o2[:, 0])
                                    nc.scalar.copy(out=dst[:, 1], in_=o2[:, 1])
                                elif ceng == 'act':
                                    nc.scalar.copy(out=dst, in_=o2[:])
                                else:
                                    nc.vector.tensor_copy(out=dst, in_=o2[:])
                            if copy_defer:
                                pend_copies.append(emit_copy)
                            else:
                                emit_copy()

                        if store_pair and pp == 1:
                            nc.sync.dma_start(
                                out=out[bh, :, 4 * mt - 4:4 * mt + 4, :],
                                in_=stage[:])
                        elif not store_pair and mt % 4 == 3:
                            ostage = stage
                            def emit_store(bh=bh, mt=mt, ostage=ostage):
                                nc.sync.dma_start(
                                    out=out[bh, :, 4 * mt - 12:4 * mt + 4, :],
                                    in_=ostage[:])
                            if copy_defer:
                                pend_stores.append(emit_store)
                            else:
                                emit_store()
            for fn in pend_copies:
                fn()
            for fn in pend_stores:
                fn()
    nc.compile()
    return nc


def kernel(q, k, v, alpha, _trace=False):
    import ml_dtypes
    from concourse.bass_utils import run_bass_kernel_spmd

    q = np.ascontiguousarray(np.asarray(q, dtype=np.float32))
    k = np.ascontiguousarray(np.asarray(k, dtype=np.float32))
    v = np.ascontiguousarray(np.asarray(v, dtype=np.float32))
    alpha = np.asarray(alpha, dtype=np.float32)

    aw = np.exp(alpha - alpha.max())
    w = aw / aw.sum()

    if "nc" not in _CACHE:
        _CACHE["nc"] = _build()
    nc = _CACHE["nc"]

    ut, ufa, ufb, uqw2 = _make_consts(w, ml_dtypes)
    bf16 = ml_dtypes.bfloat16
    # [2, pairs, 128=(m,d), S]: pre-cast bf16, pair-interleaved, transposed
    qr = q.reshape(B * H // 2, 2, S, D)
    kr = k.reshape(B * H // 2, 2, S, D)
    qk = np.stack([qr, kr], axis=0).transpose(0, 1, 2, 4, 3).reshape(
        2, B * H // 2, 128, S).astype(bf16)
    # [bh, 128, chunk, d]: partition-major V
    vt = v.reshape(B * H, NCHUNK, 128, D).transpose(0, 2, 1, 3).astype(bf16)
    in_maps = []
    for c in range(N_CORES):
        in_maps.append({
            "qk_t": np.ascontiguousarray(
                qk[:, N_PAIRS * c:N_PAIRS * (c + 1)]),
            "v_t": np.ascontiguousarray(
                vt[BH_PER_CORE * c:BH_PER_CORE * (c + 1)]),
            "ut": ut, "ufa": ufa, "ufb": ufb, "uqw2": uqw2,
        })
    res = run_bass_kernel_spmd(nc, in_maps, core_ids=list(range(N_CORES)),
                               trace=_trace)
    outs = [res.results[c]["out"] for c in range(N_CORES)]
    # [BH, 128, chunk, d] bf16 -> [B, H, S, D] f32
    full = np.concatenate(outs, axis=0).astype(np.float32)
    full = full.transpose(0, 2, 1, 3).reshape(B, H, S, D)
    if _trace:
        kernel._last_results = res
    return full
